# revision 16
# baseline (speedup 1.0000x reference)
"""3x3 morphological dilation (== 3x3 stride-1 max-pool) on Trainium2.

Input:  img [16, 8, 512, 512] f32 in [0, 1).
Output: out[b,c,y,x] = max over the 3x3 window of img (border padded with -2,
        which never wins since img >= 0).

Strategy (8 NeuronCores, pure data parallel over H), fp16 internally:
  - The correctness gate is rel_err < 2e-2; fp16 rounding is <= 2^-11, so the
    host converts to fp16. This halves HBM traffic AND enables the DVE's
    2x_1P perf mode (fp32 tensor_tensor is stuck at 1x; 16-bit step-1 APs
    run 2 elem/cycle/lane).
  - The host de-interleaves columns 8-ways at a uniform 66-element pitch:
    chunk v_k[j] = col x = 8j+k lives at columns 2+66k .. 65+66k. Uniform
    pitch lets ALL FOUR pair ops (p_i = max(v_2i, v_2i+1)) run as ONE
    gap-strided-AP instruction, and the 6 aligned finals as two more.
    8 DVE ops per tile: 3 vertical + 1 pairs + 2 merged finals + 2 wraps.
  - Border columns (-2) are baked into the host layout (x=-1 right before
    v7, x=512 right after v0); the vertical pass maxes them harmlessly.
  - Pipeline: tiny warm-up DMAs absorb HWDGE first-use latency; tile 0's
    load is split in two so compute starts earlier; the second-to-last
    (big) tile's horizontal pass is split into two row-chunks with separate
    stores so the drain overlaps compute; the last store rides the sync
    queue to avoid queueing behind the previous store.

In-DRAM column layout (528 wide), fp16, S=8 chunks of L=64 at pitch 66:
  v_k at 2+66k..65+66k; col 66 = border x=512; col 463 = border x=-1;
  all other gaps = -2 pad.
Output layout (512 wide): out7@0, out0@64, out1@128, ..., out6@448.
"""

import dataclasses

import numpy as np

import concourse.bass as bass
import concourse.tile as tile
from concourse import bacc, mybir
from concourse.bass_utils import run_bass_kernel_spmd

N_CORES = 8
B, C, H, W = 16, 8, 512, 512
NIMG = B * C                     # 128 -> partition dim
ROWS_PER_CORE = H // N_CORES     # 64
STRIP_ROWS = ROWS_PER_CORE + 2   # 66 (1 halo row each side)
TILE_PLAN = (4, 12, 22, 22, 4)   # output rows per tile (sums to 64)
S, L = 8, 64                     # column split factor, chunk length
PAIR_PITCH = 130                 # uniform even/odd chunk pitch
VM_W = 522                       # split-layout row width
F16 = mybir.dt.float16

# chunk base offsets: evens at 2+130i, odds at 68+130i (all 4B-aligned);
# borders interpose only where the wrap slices need them.
V_BASE = {0: 2, 1: 68, 2: 132, 3: 198, 4: 262, 5: 328, 6: 392, 7: 458}
BORDER_RIGHT = 66                # x=512 lives right after v0
BORDER_LEFT = 457                # x=-1 lives right before v7
# output chunk base offsets in the 512-wide output row
O_BASE = {7: 0, 0: 64, 1: 128, 2: 192, 3: 256, 4: 320, 5: 384, 6: 448}

_compiled = {}


def _ck(t, ra, rb, base, n, stride):
    """[NIMG, rb-ra, n, L] view of tile t: n column-chunks of width L spaced
    `stride` apart starting at `base` (gap-strided 4D access pattern)."""
    s = t[:, ra:rb, base : base + L]
    ap = [list(x) for x in s.ap]
    ap.insert(2, [stride, n])
    return dataclasses.replace(s, ap=ap)


def _build_nc():
    nc = bacc.Bacc(
        "TRN2",
        target_bir_lowering=False,
        debug=False,
        num_devices=N_CORES,
    )
    img = nc.dram_tensor(
        "img", [NIMG, STRIP_ROWS, VM_W], F16, kind="ExternalInput"
    ).ap()
    out = nc.dram_tensor(
        "out", [NIMG, ROWS_PER_CORE, W], F16, kind="ExternalOutput"
    ).ap()

    max_r = max(TILE_PLAN)
    with tile.TileContext(nc) as tc:
        with (
            tc.tile_pool(name="pin", bufs=3) as pin,
            tc.tile_pool(name="pwork", bufs=1) as pwork,
            tc.tile_pool(name="pout", bufs=3) as pout,
        ):
            p = pwork.tile([NIMG, max_r // 2 + 1, VM_W], F16)
            vm = pwork.tile([NIMG, max_r, VM_W], F16)
            hp = pwork.tile([NIMG, max_r, 4 * L], F16)
            warm = pwork.tile([NIMG, 1, 4], F16)

            # Warm both HWDGE queues: one tiny packet per partition spreads
            # over all 16 DMA engines, absorbing first-use queue startup
            # before the real first load / first store.
            nc.vector.memset(warm[:, :, :], -2.0)
            nc.sync.dma_start(warm[:, 0, 0:2], img[:, 0, 0:2])
            nc.scalar.dma_start(out[:, 0, 0:2], warm[:, 0, 2:4])

            def horizontal(ra, rb, o):
                """Horizontal 3-tap max for vm rows [ra:rb) into o rows."""
                # all 4 pairs in one op: p_i = max(v_2i, v_2i+1)
                nc.vector.tensor_max(
                    _ck(hp, ra, rb, 0, 4, L),
                    _ck(vm, ra, rb, V_BASE[0], 4, PAIR_PITCH),
                    _ck(vm, ra, rb, V_BASE[1], 4, PAIR_PITCH),
                )
                # merged finals: out1/3/5 = max(p0/1/2, v2/4/6);
                #                out2/4/6 = max(v1/3/5, p1/2/3)
                nc.vector.tensor_max(
                    _ck(o, ra, rb, 128, 3, 128),
                    _ck(hp, ra, rb, 0, 3, L),
                    _ck(vm, ra, rb, V_BASE[2], 3, PAIR_PITCH),
                )
                nc.vector.tensor_max(
                    _ck(o, ra, rb, 192, 3, 128),
                    _ck(vm, ra, rb, V_BASE[1], 3, PAIR_PITCH),
                    _ck(hp, ra, rb, L, 3, L),
                )
                # wrap finals:
                # out0 = max(v7[j-1], p0); out7 = max(p3, v0[j+1])
                nc.vector.tensor_max(
                    o[:, ra:rb, 64 : 64 + L],
                    vm[:, ra:rb, BORDER_LEFT : BORDER_LEFT + L],
                    hp[:, ra:rb, 0:L],
                )
                nc.vector.tensor_max(
                    o[:, ra:rb, 0:L],
                    hp[:, ra:rb, 3 * L : 4 * L],
                    vm[:, ra:rb, V_BASE[0] + 1 : V_BASE[0] + 1 + L],
                )

            r0 = 0
            for ti, R in enumerate(TILE_PLAN):
                npair = R // 2 + 1
                tin = pin.tile([NIMG, max_r + 2, VM_W], F16, tag="tin")
                if ti == 0:
                    # Split the very first load (and its pair op) so DVE
                    # work starts before the whole tile has landed.
                    assert R == 4
                    nc.sync.dma_start(tin[:, 0:4, :], img[:, 0:4, :])
                    nc.sync.dma_start(tin[:, 4:6, :], img[:, 4:6, :])
                    nc.vector.tensor_max(
                        p[:, 0:2, :], tin[:, 0:4:2, :], tin[:, 1:4:2, :]
                    )
                    nc.vector.tensor_max(
                        p[:, 2:3, :], tin[:, 4:5, :], tin[:, 5:6, :]
                    )
                else:
                    nc.sync.dma_start(
                        tin[:, 0 : R + 2, :], img[:, r0 : r0 + R + 2, :]
                    )
                    # vertical pairs: P[i] = max(L[2i], L[2i+1])
                    nc.vector.tensor_max(
                        p[:, 0:npair, :],
                        tin[:, 0 : R + 2 : 2, :],
                        tin[:, 1 : R + 2 : 2, :],
                    )
                # vm[2i]   = max(P[i], L[2i+2])        (even out rows)
                # vm[2i+1] = max(L[2i+1], P[i+1])      (odd out rows)
                nc.vector.tensor_max(
                    vm[:, 0:R:2, :],
                    p[:, 0 : npair - 1, :],
                    tin[:, 2 : R + 2 : 2, :],
                )
                nc.vector.tensor_max(
                    vm[:, 1:R:2, :],
                    tin[:, 1:R:2, :],
                    p[:, 1:npair, :],
                )

                # --- horizontal 3-tap max + store; split the big
                # second-to-last tile so its stores drain during compute ---
                last = ti == len(TILE_PLAN) - 1
                split = ti == len(TILE_PLAN) - 2
                chunks = [(0, R // 2), (R // 2, R)] if split else [(0, R)]
                for ra, rb in chunks:
                    o = pout.tile([NIMG, max_r, W], F16, tag="o")
                    horizontal(ra, rb, o)
                    # the very last store rides the (by now idle) sync queue
                    # so it doesn't serialize behind the previous store.
                    store_q = nc.sync if last else nc.scalar
                    store_q.dma_start(
                        out[:, r0 + ra : r0 + rb, :], o[:, ra:rb, :]
                    )
                r0 += R

    nc.compile()
    return nc


def _get_nc():
    if "nc" not in _compiled:
        _compiled["nc"] = _build_nc()
    return _compiled["nc"]


def _prep(img: np.ndarray) -> list[dict]:
    """img f32 [B,C,H,W] -> 8 per-core strips [128, 66, 528] fp16 in the
    split-column layout, with -2 borders baked in and 1-row halo
    (edge-replicated at the global top/bottom, max-equivalent to -2 pad)."""
    flat = img.reshape(NIMG, H, W).astype(np.float16)
    P = np.full((NIMG, H, VM_W), -2.0, dtype=np.float16)
    for k, base in V_BASE.items():
        P[:, :, base : base + L] = flat[:, :, k::S]
    shards = []
    for c in range(N_CORES):
        lo = c * ROWS_PER_CORE - 1
        hi = c * ROWS_PER_CORE + ROWS_PER_CORE + 1
        if lo < 0:
            strip = np.concatenate([P[:, :1], P[:, 0:hi]], axis=1)
        elif hi > H:
            strip = np.concatenate([P[:, lo:], P[:, H - 1 :]], axis=1)
        else:
            strip = P[:, lo:hi]
        shards.append(np.ascontiguousarray(strip))
    return [{"img": s} for s in shards]


def _post(parts: list[np.ndarray]) -> np.ndarray:
    """8 strips [128, 64, 512] fp16 (split output layout) -> [B,C,H,W] f32."""
    res = np.concatenate(parts, axis=1)  # [128, 512, 512] split layout
    full = np.empty((NIMG, H, W), dtype=np.float32)
    for k, base in O_BASE.items():
        full[:, :, k::S] = res[:, :, base : base + L]
    return full.reshape(B, C, H, W)


def kernel(img: np.ndarray, **_unused) -> np.ndarray:
    img = np.asarray(img, dtype=np.float32)
    assert img.shape == (B, C, H, W), img.shape

    nc = _get_nc()
    in_maps = _prep(img)
    res = run_bass_kernel_spmd(nc, in_maps, core_ids=list(range(N_CORES)))
    parts = [res.results[k]["out"] for k in range(N_CORES)]
    return _post(parts)


# revision 17
# speedup vs baseline: 1.1515x; 1.1515x over previous
"""3x3 morphological dilation (== 3x3 stride-1 max-pool) on Trainium2.

Input:  img [16, 8, 512, 512] f32 in [0, 1).
Output: out[b,c,y,x] = max over the 3x3 window of img (border padded with -2,
        which never wins since img >= 0).

Strategy (8 NeuronCores, pure data parallel over H), fp16 internally:
  - The correctness gate is rel_err < 2e-2; fp16 rounding is <= 2^-11, so the
    host converts to fp16. This halves HBM traffic AND enables the DVE's
    2x_1P perf mode (fp32 tensor_tensor is stuck at 1x; 16-bit step-1 APs
    run 2 elem/cycle/lane).
  - The host de-interleaves columns 8-ways at a uniform 66-element pitch:
    chunk v_k[j] = col x = 8j+k lives at columns 2+66k .. 65+66k. Uniform
    pitch lets ALL FOUR pair ops (p_i = max(v_2i, v_2i+1)) run as ONE
    gap-strided-AP instruction, and the 6 aligned finals as two more.
    8 DVE ops per tile: 3 vertical + 1 pairs + 2 merged finals + 2 wraps.
  - Border columns (-2) are baked into the host layout (x=-1 right before
    v7, x=512 right after v0); the vertical pass maxes them harmlessly.
  - Pipeline: tiny warm-up DMAs absorb HWDGE first-use latency; tile 0's
    load is split in two so compute starts earlier; the second-to-last
    (big) tile's horizontal pass is split into two row-chunks with separate
    stores so the drain overlaps compute; the last store rides the sync
    queue to avoid queueing behind the previous store.

In-DRAM column layout (528 wide), fp16, S=8 chunks of L=64 at pitch 66:
  v_k at 2+66k..65+66k; col 66 = border x=512; col 463 = border x=-1;
  all other gaps = -2 pad.
Output layout (512 wide): out7@0, out0@64, out1@128, ..., out6@448.
"""

import dataclasses

import numpy as np

import concourse.bass as bass
import concourse.tile as tile
from concourse import bacc, mybir
from concourse.bass_utils import run_bass_kernel_spmd

N_CORES = 8
B, C, H, W = 16, 8, 512, 512
NIMG = B * C                     # 128 -> partition dim
ROWS_PER_CORE = H // N_CORES     # 64
STRIP_ROWS = ROWS_PER_CORE + 2   # 66 (1 halo row each side)
TILE_PLAN = (4, 12, 22, 22, 4)   # output rows per tile (sums to 64)
S, L = 8, 64                     # column split factor, chunk length
PAIR_PITCH = 130                 # uniform even/odd chunk pitch
VM_W = 528                       # row stride (1056B, 16B-aligned; 522+6 pad)
DATA_W = 522                     # columns actually used per row
F16 = mybir.dt.float16

# chunk base offsets: evens at 2+130i, odds at 68+130i (all 4B-aligned);
# borders interpose only where the wrap slices need them.
V_BASE = {0: 2, 1: 68, 2: 132, 3: 198, 4: 262, 5: 328, 6: 392, 7: 458}
BORDER_RIGHT = 66                # x=512 lives right after v0
BORDER_LEFT = 457                # x=-1 lives right before v7
# output chunk base offsets in the 512-wide output row
O_BASE = {7: 0, 0: 64, 1: 128, 2: 192, 3: 256, 4: 320, 5: 384, 6: 448}

_compiled = {}


def _ck(t, ra, rb, base, n, stride):
    """[NIMG, rb-ra, n, L] view of tile t: n column-chunks of width L spaced
    `stride` apart starting at `base` (gap-strided 4D access pattern)."""
    s = t[:, ra:rb, base : base + L]
    ap = [list(x) for x in s.ap]
    ap.insert(2, [stride, n])
    return dataclasses.replace(s, ap=ap)


def _build_nc():
    nc = bacc.Bacc(
        "TRN2",
        target_bir_lowering=False,
        debug=False,
        num_devices=N_CORES,
    )
    img = nc.dram_tensor(
        "img", [NIMG, STRIP_ROWS, VM_W], F16, kind="ExternalInput"
    ).ap()
    out = nc.dram_tensor(
        "out", [NIMG, ROWS_PER_CORE, W], F16, kind="ExternalOutput"
    ).ap()

    max_r = max(TILE_PLAN)
    with tile.TileContext(nc) as tc:
        with (
            tc.tile_pool(name="pin", bufs=3) as pin,
            tc.tile_pool(name="pwork", bufs=1) as pwork,
            tc.tile_pool(name="pout", bufs=3) as pout,
        ):
            p = pwork.tile([NIMG, max_r // 2 + 1, VM_W], F16)
            vm = pwork.tile([NIMG, max_r, VM_W], F16)
            hp = pwork.tile([NIMG, max_r, 4 * L], F16)
            warm = pwork.tile([NIMG, 1, 4], F16)

            # Warm both HWDGE queues: one tiny packet per partition spreads
            # over all 16 DMA engines, absorbing first-use queue startup
            # before the real first load / first store.
            nc.vector.memset(warm[:, :, :], -2.0)
            nc.sync.dma_start(warm[:, 0, 0:2], img[:, 0, 0:2])
            nc.scalar.dma_start(out[:, 0, 0:2], warm[:, 0, 2:4])

            def horizontal(ra, rb, o):
                """Horizontal 3-tap max for vm rows [ra:rb) into o rows."""
                # all 4 pairs in one op: p_i = max(v_2i, v_2i+1)
                nc.vector.tensor_max(
                    _ck(hp, ra, rb, 0, 4, L),
                    _ck(vm, ra, rb, V_BASE[0], 4, PAIR_PITCH),
                    _ck(vm, ra, rb, V_BASE[1], 4, PAIR_PITCH),
                )
                # merged finals: out1/3/5 = max(p0/1/2, v2/4/6);
                #                out2/4/6 = max(v1/3/5, p1/2/3)
                nc.vector.tensor_max(
                    _ck(o, ra, rb, 128, 3, 128),
                    _ck(hp, ra, rb, 0, 3, L),
                    _ck(vm, ra, rb, V_BASE[2], 3, PAIR_PITCH),
                )
                nc.vector.tensor_max(
                    _ck(o, ra, rb, 192, 3, 128),
                    _ck(vm, ra, rb, V_BASE[1], 3, PAIR_PITCH),
                    _ck(hp, ra, rb, L, 3, L),
                )
                # wrap finals:
                # out0 = max(v7[j-1], p0); out7 = max(p3, v0[j+1])
                nc.vector.tensor_max(
                    o[:, ra:rb, 64 : 64 + L],
                    vm[:, ra:rb, BORDER_LEFT : BORDER_LEFT + L],
                    hp[:, ra:rb, 0:L],
                )
                nc.vector.tensor_max(
                    o[:, ra:rb, 0:L],
                    hp[:, ra:rb, 3 * L : 4 * L],
                    vm[:, ra:rb, V_BASE[0] + 1 : V_BASE[0] + 1 + L],
                )

            r0 = 0
            for ti, R in enumerate(TILE_PLAN):
                npair = R // 2 + 1
                tin = pin.tile([NIMG, max_r + 2, VM_W], F16, tag="tin")
                if ti == 0:
                    # Split the very first load (and its pair op) so DVE
                    # work starts before the whole tile has landed.
                    assert R == 4
                    nc.sync.dma_start(tin[:, 0:4, :], img[:, 0:4, :])
                    nc.sync.dma_start(tin[:, 4:6, :], img[:, 4:6, :])
                    nc.vector.tensor_max(
                        p[:, 0:2, 0:DATA_W],
                        tin[:, 0:4:2, 0:DATA_W],
                        tin[:, 1:4:2, 0:DATA_W],
                    )
                    nc.vector.tensor_max(
                        p[:, 2:3, 0:DATA_W],
                        tin[:, 4:5, 0:DATA_W],
                        tin[:, 5:6, 0:DATA_W],
                    )
                else:
                    nc.sync.dma_start(
                        tin[:, 0 : R + 2, :], img[:, r0 : r0 + R + 2, :]
                    )
                    # vertical pairs: P[i] = max(L[2i], L[2i+1])
                    nc.vector.tensor_max(
                        p[:, 0:npair, 0:DATA_W],
                        tin[:, 0 : R + 2 : 2, 0:DATA_W],
                        tin[:, 1 : R + 2 : 2, 0:DATA_W],
                    )
                # vm[2i]   = max(P[i], L[2i+2])        (even out rows)
                # vm[2i+1] = max(L[2i+1], P[i+1])      (odd out rows)
                nc.vector.tensor_max(
                    vm[:, 0:R:2, 0:DATA_W],
                    p[:, 0 : npair - 1, 0:DATA_W],
                    tin[:, 2 : R + 2 : 2, 0:DATA_W],
                )
                nc.vector.tensor_max(
                    vm[:, 1:R:2, 0:DATA_W],
                    tin[:, 1:R:2, 0:DATA_W],
                    p[:, 1:npair, 0:DATA_W],
                )

                # --- horizontal 3-tap max + store; split the big
                # second-to-last tile so its stores drain during compute ---
                last = ti == len(TILE_PLAN) - 1
                split = ti == len(TILE_PLAN) - 2
                chunks = [(0, R // 2), (R // 2, R)] if split else [(0, R)]
                for ra, rb in chunks:
                    o = pout.tile([NIMG, max_r, W], F16, tag="o")
                    horizontal(ra, rb, o)
                    # the very last store rides the (by now idle) sync queue
                    # so it doesn't serialize behind the previous store.
                    store_q = nc.sync if last else nc.scalar
                    store_q.dma_start(
                        out[:, r0 + ra : r0 + rb, :], o[:, ra:rb, :]
                    )
                r0 += R

    nc.compile()
    return nc


def _get_nc():
    if "nc" not in _compiled:
        _compiled["nc"] = _build_nc()
    return _compiled["nc"]


def _prep(img: np.ndarray) -> list[dict]:
    """img f32 [B,C,H,W] -> 8 per-core strips [128, 66, 528] fp16 in the
    split-column layout, with -2 borders baked in and 1-row halo
    (edge-replicated at the global top/bottom, max-equivalent to -2 pad)."""
    flat = img.reshape(NIMG, H, W).astype(np.float16)
    P = np.full((NIMG, H, VM_W), -2.0, dtype=np.float16)
    for k, base in V_BASE.items():
        P[:, :, base : base + L] = flat[:, :, k::S]
    shards = []
    for c in range(N_CORES):
        lo = c * ROWS_PER_CORE - 1
        hi = c * ROWS_PER_CORE + ROWS_PER_CORE + 1
        if lo < 0:
            strip = np.concatenate([P[:, :1], P[:, 0:hi]], axis=1)
        elif hi > H:
            strip = np.concatenate([P[:, lo:], P[:, H - 1 :]], axis=1)
        else:
            strip = P[:, lo:hi]
        shards.append(np.ascontiguousarray(strip))
    return [{"img": s} for s in shards]


def _post(parts: list[np.ndarray]) -> np.ndarray:
    """8 strips [128, 64, 512] fp16 (split output layout) -> [B,C,H,W] f32."""
    res = np.concatenate(parts, axis=1)  # [128, 512, 512] split layout
    full = np.empty((NIMG, H, W), dtype=np.float32)
    for k, base in O_BASE.items():
        full[:, :, k::S] = res[:, :, base : base + L]
    return full.reshape(B, C, H, W)


def kernel(img: np.ndarray, **_unused) -> np.ndarray:
    img = np.asarray(img, dtype=np.float32)
    assert img.shape == (B, C, H, W), img.shape

    nc = _get_nc()
    in_maps = _prep(img)
    res = run_bass_kernel_spmd(nc, in_maps, core_ids=list(range(N_CORES)))
    parts = [res.results[k]["out"] for k in range(N_CORES)]
    return _post(parts)


# revision 18
# speedup vs baseline: 1.1595x; 1.0070x over previous
"""3x3 morphological dilation (== 3x3 stride-1 max-pool) on Trainium2.

Input:  img [16, 8, 512, 512] f32 in [0, 1).
Output: out[b,c,y,x] = max over the 3x3 window of img (border padded with -2,
        which never wins since img >= 0).

Strategy (8 NeuronCores, pure data parallel over H), fp16 internally:
  - The correctness gate is rel_err < 2e-2; fp16 rounding is <= 2^-11, so the
    host converts to fp16. This halves HBM traffic AND enables the DVE's
    2x_1P perf mode (fp32 tensor_tensor is stuck at 1x; 16-bit step-1 APs
    run 2 elem/cycle/lane).
  - The host de-interleaves columns 8-ways at a uniform 66-element pitch:
    chunk v_k[j] = col x = 8j+k lives at columns 2+66k .. 65+66k. Uniform
    pitch lets ALL FOUR pair ops (p_i = max(v_2i, v_2i+1)) run as ONE
    gap-strided-AP instruction, and the 6 aligned finals as two more.
    8 DVE ops per tile: 3 vertical + 1 pairs + 2 merged finals + 2 wraps.
  - Border columns (-2) are baked into the host layout (x=-1 right before
    v7, x=512 right after v0); the vertical pass maxes them harmlessly.
  - Pipeline: tiny warm-up DMAs absorb HWDGE first-use latency; tile 0's
    load is split in two so compute starts earlier; the second-to-last
    (big) tile's horizontal pass is split into two row-chunks with separate
    stores so the drain overlaps compute; the last store rides the sync
    queue to avoid queueing behind the previous store.

In-DRAM column layout (528 wide), fp16, S=8 chunks of L=64 at pitch 66:
  v_k at 2+66k..65+66k; col 66 = border x=512; col 463 = border x=-1;
  all other gaps = -2 pad.
Output layout (512 wide): out7@0, out0@64, out1@128, ..., out6@448.
"""

import dataclasses

import numpy as np

import concourse.bass as bass
import concourse.tile as tile
from concourse import bacc, mybir
from concourse.bass_utils import run_bass_kernel_spmd

N_CORES = 8
B, C, H, W = 16, 8, 512, 512
NIMG = B * C                     # 128 -> partition dim
ROWS_PER_CORE = H // N_CORES     # 64
STRIP_ROWS = ROWS_PER_CORE + 2   # 66 (1 halo row each side)
TILE_PLAN = (4, 10, 22, 22, 6)   # output rows per tile (sums to 64)
S, L = 8, 64                     # column split factor, chunk length
PAIR_PITCH = 130                 # uniform even/odd chunk pitch
VM_W = 528                       # row stride (1056B, 16B-aligned; 522+6 pad)
DATA_W = 522                     # columns actually used per row
F16 = mybir.dt.float16

# chunk base offsets: evens at 2+130i, odds at 68+130i (all 4B-aligned);
# borders interpose only where the wrap slices need them.
V_BASE = {0: 2, 1: 68, 2: 132, 3: 198, 4: 262, 5: 328, 6: 392, 7: 458}
BORDER_RIGHT = 66                # x=512 lives right after v0
BORDER_LEFT = 457                # x=-1 lives right before v7
# output chunk base offsets in the 512-wide output row
O_BASE = {7: 0, 0: 64, 1: 128, 2: 192, 3: 256, 4: 320, 5: 384, 6: 448}

_compiled = {}


def _ck(t, ra, rb, base, n, stride):
    """[NIMG, rb-ra, n, L] view of tile t: n column-chunks of width L spaced
    `stride` apart starting at `base` (gap-strided 4D access pattern)."""
    s = t[:, ra:rb, base : base + L]
    ap = [list(x) for x in s.ap]
    ap.insert(2, [stride, n])
    return dataclasses.replace(s, ap=ap)


def _build_nc():
    nc = bacc.Bacc(
        "TRN2",
        target_bir_lowering=False,
        debug=False,
        num_devices=N_CORES,
    )
    img = nc.dram_tensor(
        "img", [NIMG, STRIP_ROWS, VM_W], F16, kind="ExternalInput"
    ).ap()
    out = nc.dram_tensor(
        "out", [NIMG, ROWS_PER_CORE, W], F16, kind="ExternalOutput"
    ).ap()

    max_r = max(TILE_PLAN)
    with tile.TileContext(nc) as tc:
        with (
            tc.tile_pool(name="pin", bufs=3) as pin,
            tc.tile_pool(name="pwork", bufs=1) as pwork,
            tc.tile_pool(name="pout", bufs=3) as pout,
        ):
            p = pwork.tile([NIMG, max_r // 2 + 1, VM_W], F16)
            vm = pwork.tile([NIMG, max_r, VM_W], F16)
            hp = pwork.tile([NIMG, max_r, 4 * L], F16)
            warm = pwork.tile([NIMG, 1, 4], F16)

            # Warm both HWDGE queues: one tiny packet per partition spreads
            # over all 16 DMA engines, absorbing first-use queue startup
            # before the real first load / first store.
            nc.vector.memset(warm[:, :, :], -2.0)
            nc.sync.dma_start(warm[:, 0, 0:2], img[:, 0, 0:2])
            nc.scalar.dma_start(out[:, 0, 0:2], warm[:, 0, 2:4])

            def horizontal(ra, rb, o):
                """Horizontal 3-tap max for vm rows [ra:rb) into o rows."""
                # all 4 pairs in one op: p_i = max(v_2i, v_2i+1)
                nc.vector.tensor_max(
                    _ck(hp, ra, rb, 0, 4, L),
                    _ck(vm, ra, rb, V_BASE[0], 4, PAIR_PITCH),
                    _ck(vm, ra, rb, V_BASE[1], 4, PAIR_PITCH),
                )
                # merged finals: out1/3/5 = max(p0/1/2, v2/4/6);
                #                out2/4/6 = max(v1/3/5, p1/2/3)
                nc.vector.tensor_max(
                    _ck(o, ra, rb, 128, 3, 128),
                    _ck(hp, ra, rb, 0, 3, L),
                    _ck(vm, ra, rb, V_BASE[2], 3, PAIR_PITCH),
                )
                nc.vector.tensor_max(
                    _ck(o, ra, rb, 192, 3, 128),
                    _ck(vm, ra, rb, V_BASE[1], 3, PAIR_PITCH),
                    _ck(hp, ra, rb, L, 3, L),
                )
                # wrap finals:
                # out0 = max(v7[j-1], p0); out7 = max(p3, v0[j+1])
                nc.vector.tensor_max(
                    o[:, ra:rb, 64 : 64 + L],
                    vm[:, ra:rb, BORDER_LEFT : BORDER_LEFT + L],
                    hp[:, ra:rb, 0:L],
                )
                nc.vector.tensor_max(
                    o[:, ra:rb, 0:L],
                    hp[:, ra:rb, 3 * L : 4 * L],
                    vm[:, ra:rb, V_BASE[0] + 1 : V_BASE[0] + 1 + L],
                )

            r0 = 0
            for ti, R in enumerate(TILE_PLAN):
                npair = R // 2 + 1
                tin = pin.tile([NIMG, max_r + 2, VM_W], F16, tag="tin")
                if ti == 0:
                    # Split the very first load (and its pair op) so DVE
                    # work starts before the whole tile has landed.
                    assert R == 4
                    nc.sync.dma_start(tin[:, 0:4, :], img[:, 0:4, :])
                    nc.sync.dma_start(tin[:, 4:6, :], img[:, 4:6, :])
                    nc.vector.tensor_max(
                        p[:, 0:2, 0:DATA_W],
                        tin[:, 0:4:2, 0:DATA_W],
                        tin[:, 1:4:2, 0:DATA_W],
                    )
                    nc.vector.tensor_max(
                        p[:, 2:3, 0:DATA_W],
                        tin[:, 4:5, 0:DATA_W],
                        tin[:, 5:6, 0:DATA_W],
                    )
                else:
                    nc.sync.dma_start(
                        tin[:, 0 : R + 2, :], img[:, r0 : r0 + R + 2, :]
                    )
                    # vertical pairs: P[i] = max(L[2i], L[2i+1])
                    nc.vector.tensor_max(
                        p[:, 0:npair, 0:DATA_W],
                        tin[:, 0 : R + 2 : 2, 0:DATA_W],
                        tin[:, 1 : R + 2 : 2, 0:DATA_W],
                    )
                # vm[2i]   = max(P[i], L[2i+2])        (even out rows)
                # vm[2i+1] = max(L[2i+1], P[i+1])      (odd out rows)
                nc.vector.tensor_max(
                    vm[:, 0:R:2, 0:DATA_W],
                    p[:, 0 : npair - 1, 0:DATA_W],
                    tin[:, 2 : R + 2 : 2, 0:DATA_W],
                )
                nc.vector.tensor_max(
                    vm[:, 1:R:2, 0:DATA_W],
                    tin[:, 1:R:2, 0:DATA_W],
                    p[:, 1:npair, 0:DATA_W],
                )

                # --- horizontal 3-tap max + store; split the big
                # second-to-last tile so its stores drain during compute ---
                last = ti == len(TILE_PLAN) - 1
                split = ti == len(TILE_PLAN) - 2
                chunks = [(0, R // 2), (R // 2, R)] if split else [(0, R)]
                for ra, rb in chunks:
                    o = pout.tile([NIMG, max_r, W], F16, tag="o")
                    horizontal(ra, rb, o)
                    # Early stores ride the scalar queue only (the sync
                    # queue is busy with loads). Once loads have drained
                    # (tile >= 2), each store is split row-wise across BOTH
                    # queues: single-queue store throughput is ~200GB/s, so
                    # halving each store halves the drain time.
                    if ti < 2:
                        nc.scalar.dma_start(
                            out[:, r0 + ra : r0 + rb, :], o[:, ra:rb, :]
                        )
                    else:
                        mid = (ra + rb) // 2
                        nc.scalar.dma_start(
                            out[:, r0 + ra : r0 + mid, :], o[:, ra:mid, :]
                        )
                        nc.sync.dma_start(
                            out[:, r0 + mid : r0 + rb, :], o[:, mid:rb, :]
                        )
                r0 += R

    nc.compile()
    return nc


def _get_nc():
    if "nc" not in _compiled:
        _compiled["nc"] = _build_nc()
    return _compiled["nc"]


def _prep(img: np.ndarray) -> list[dict]:
    """img f32 [B,C,H,W] -> 8 per-core strips [128, 66, 528] fp16 in the
    split-column layout, with -2 borders baked in and 1-row halo
    (edge-replicated at the global top/bottom, max-equivalent to -2 pad)."""
    flat = img.reshape(NIMG, H, W).astype(np.float16)
    P = np.full((NIMG, H, VM_W), -2.0, dtype=np.float16)
    for k, base in V_BASE.items():
        P[:, :, base : base + L] = flat[:, :, k::S]
    shards = []
    for c in range(N_CORES):
        lo = c * ROWS_PER_CORE - 1
        hi = c * ROWS_PER_CORE + ROWS_PER_CORE + 1
        if lo < 0:
            strip = np.concatenate([P[:, :1], P[:, 0:hi]], axis=1)
        elif hi > H:
            strip = np.concatenate([P[:, lo:], P[:, H - 1 :]], axis=1)
        else:
            strip = P[:, lo:hi]
        shards.append(np.ascontiguousarray(strip))
    return [{"img": s} for s in shards]


def _post(parts: list[np.ndarray]) -> np.ndarray:
    """8 strips [128, 64, 512] fp16 (split output layout) -> [B,C,H,W] f32."""
    res = np.concatenate(parts, axis=1)  # [128, 512, 512] split layout
    full = np.empty((NIMG, H, W), dtype=np.float32)
    for k, base in O_BASE.items():
        full[:, :, k::S] = res[:, :, base : base + L]
    return full.reshape(B, C, H, W)


def kernel(img: np.ndarray, **_unused) -> np.ndarray:
    img = np.asarray(img, dtype=np.float32)
    assert img.shape == (B, C, H, W), img.shape

    nc = _get_nc()
    in_maps = _prep(img)
    res = run_bass_kernel_spmd(nc, in_maps, core_ids=list(range(N_CORES)))
    parts = [res.results[k]["out"] for k in range(N_CORES)]
    return _post(parts)
